# revision 6
# baseline (speedup 1.0000x reference)
"""ContinuousTimeRNN Trainium2 kernel (v2: PSUM-resident h, bf16 matmuls).

Data-parallel over batch N=512 across 8 NeuronCores (NS=64 rows each).
h is kept transposed (H on partitions, [128, 4, 64]) and lives in PSUM.
Per step, one PSUM accumulation group computes

    h' = (0.9*I) @ h  +  sum_k (0.1*W_rec)[k].T @ a[k]  +  (0.1*[W_in;b]).T @ x3

where the 0.9*I matmul (fp32r, free=256 -> 1 cyc/row) opens the group
(start=True sets the PSUM has_written bits) and the 16 W_rec chunk matmuls
plus 4 x-chunk matmuls (bf16 moving tensors -> 1 cyc/row at any p-state)
accumulate on top.  The activation path reads PSUM directly:
ACT tanh(psum)->bf16, DVE max(.,0)->bf16, and DVE copies h to an SBUF
history ring (50 slots) that feeds both the 0.9*I matmul and the y output
matmuls (interleaved one per step into PE idle gaps).
"""

import sys

sys.path.insert(0, "/opt/trn_rl_repo")

import numpy as np

ALPHA = 0.1
T, N, H, DIN, DOUT, INIT = 1000, 512, 512, 2, 2, 2
NCORES = 8
NS = N // NCORES          # 64 batch rows per core
WIN = 25                  # y-flush window (steps)
BODY = 2 * WIN            # steps per For_i body
NK = H // 128             # 4 H-chunks
QW = WIN * NS // 4        # 400 columns per y quarter


def _build_nc(t_total=T, reps=1):
    import concourse.bass as bass
    import concourse.mybir as mybir
    from concourse import bacc
    from concourse.tile import TileContext
    from concourse.masks import make_identity
    from concourse.bass import ds

    fp32 = mybir.dt.float32
    fp32r = mybir.dt.float32r
    bf16 = mybir.dt.bfloat16
    AFT = mybir.ActivationFunctionType

    nc = bacc.Bacc("TRN2", target_bir_lowering=False, debug=False,
                   num_devices=NCORES)

    nbody = t_total // BODY

    # -------- DRAM I/O (per core) --------
    wrec_d = nc.dram_tensor("wrec", [NK, 128, H], bf16, kind="ExternalInput").ap()
    win3_d = nc.dram_tensor("win3", [DIN + 1, H], bf16, kind="ExternalInput").ap()
    wout_d = nc.dram_tensor("wout", [NK, 128, DOUT], bf16, kind="ExternalInput").ap()
    fcw3_d = nc.dram_tensor("fcw3", [INIT + 1, H], fp32, kind="ExternalInput").ap()
    init3_d = nc.dram_tensor("init3", [INIT + 1, NS], fp32, kind="ExternalInput").ap()
    xt_d = nc.dram_tensor("xt", [DIN + 1, t_total * NS], bf16, kind="ExternalInput").ap()
    # yA holds rows with (t % 50) < 25; yB holds the rest, shifted +BODY rows
    ya_d = nc.dram_tensor("ya", [DOUT, t_total * NS], fp32, kind="ExternalOutput").ap()
    yb_d = nc.dram_tensor("yb", [DOUT, (t_total + BODY) * NS], fp32,
                          kind="ExternalOutput").ap()

    with TileContext(nc) as tc:
        with (
            tc.tile_pool(name="wpool", bufs=1) as wpool,
            tc.tile_pool(name="hpool", bufs=1) as hpool,
            tc.tile_pool(name="apool", bufs=3) as apool,
            tc.tile_pool(name="xpool", bufs=2) as xpool,
            tc.tile_pool(name="ypool", bufs=2) as ypool,
            tc.tile_pool(name="hps", bufs=1, space="PSUM") as hpspool,
            tc.tile_pool(name="py", bufs=2, space="PSUM") as pypool,
        ):
            # -------- persistent SBUF --------
            wrec_sb = wpool.tile([128, NK, H], bf16)       # 0.1*W_rec chunks
            win3_sb = wpool.tile([DIN + 1, H], bf16)       # 0.1*[W_in; bias]
            wout_sb = wpool.tile([128, NK, DOUT], bf16)    # W_out/0.9 chunks
            fcw3_sb = wpool.tile([INIT + 1, H], fp32)      # [fc_w.T; fc_b]
            init3_sb = wpool.tile([INIT + 1, NS], fp32)    # [initdir.T; ones]
            ident = wpool.tile([128, 128], bf16)           # identity (bf16)
            # h history ring (bf16, holds 0.9*h_after): slot s-1 feeds the
            # identity opener matmul of step s and the y matmuls (W_out/0.9)
            hist = hpool.tile([128, NK, BODY * NS], bf16)

            for k in range(NK):
                nc.sync.dma_start(out=wrec_sb[:, k, :], in_=wrec_d[k])
                nc.sync.dma_start(out=wout_sb[:, k, :], in_=wout_d[k])
            nc.sync.dma_start(out=win3_sb[:], in_=win3_d)
            nc.sync.dma_start(out=fcw3_sb[:], in_=fcw3_d)
            nc.sync.dma_start(out=init3_sb[:], in_=init3_d)
            make_identity(nc, ident[:])

            # persistent PSUM h tiles (ping-pong); h0 = fc(initdir) -> hpsB
            hpsA = hpspool.tile([128, NK, NS], fp32, tag="hpsA")
            hpsB = hpspool.tile([128, NK, NS], fp32, tag="hpsB")
            for m in range(NK):
                nc.tensor.matmul(hpsB[:, m, :],
                                 fcw3_sb[:, m * 128:(m + 1) * 128],
                                 init3_sb[:], start=True, stop=True)

            # -------- time loop --------
            with tc.For_i(0, reps, 1) as _rep, \
                 tc.For_i(0, t_total, BODY) as iv:
                xbuf = xpool.tile([DIN + 1, BODY * NS], bf16)
                nc.sync.dma_start(out=xbuf[:], in_=xt_d[:, ds(iv * NS, BODY * NS)])

                ysbB = ypool.tile([DOUT, WIN * NS], fp32, tag="ysbB")
                ysbA = ypool.tile([DOUT, WIN * NS], fp32, tag="ysbA")

                # y matmuls pending per step: (ysb, colbase, q) emitted one
                # PE-accumulation-quarter (4 matmuls) per step.
                # s=1..4   : prev-body window B (hist slots 25..49)
                # s=26..29 : this-body window A (hist slots 0..24)
                for s in range(BODY):
                    cur, prev = (hpsA, hpsB) if s % 2 == 0 else (hpsB, hpsA)
                    slot = (s - 1) % BODY

                    # 0.9*h_after(s-1) -> hist ring (bf16)
                    nc.vector.tensor_scalar_mul(
                        hist[:, :, slot * NS:(slot + 1) * NS], prev[:],
                        1.0 - ALPHA)
                    # a = max(tanh(h), 0) in bf16
                    tbuf = apool.tile([128, NK * NS], bf16, tag="tbuf")
                    abuf = apool.tile([128, NK, NS], bf16, tag="abuf")
                    nc.scalar.activation(tbuf[:], prev[:].rearrange(
                        "p k n -> p (k n)"), AFT.Tanh)
                    nc.vector.tensor_scalar_max(
                        abuf[:], tbuf[:].rearrange("p (k n) -> p k n", k=NK), 0.0)

                    # h' accumulation group: 0.9*I opener + 16 aW + 4 x
                    nc.tensor.matmul(
                        cur[:],
                        ident[:],
                        hist[:, :, slot * NS:(slot + 1) * NS],
                        start=True, stop=False, skip_group_check=True)
                    for k in range(NK):
                        for m in range(NK):
                            nc.tensor.matmul(
                                cur[:, m, :],
                                wrec_sb[:, k, m * 128:(m + 1) * 128],
                                abuf[:, k, :],
                                start=False, stop=False, skip_group_check=True)
                    for m in range(NK):
                        nc.tensor.matmul(
                            cur[:, m, :],
                            win3_sb[:, m * 128:(m + 1) * 128],
                            xbuf[:, s * NS:(s + 1) * NS],
                            start=False, stop=True, skip_group_check=True)

                    # interleaved y work (one PSUM quarter per step)
                    yq = None
                    if 1 <= s <= 4:        # prev-body window B: slots 25..49
                        yq = (ysbB, WIN * NS, s - 1)
                    elif 26 <= s <= 29:    # this-body window A: slots 0..24
                        yq = (ysbA, 0, s - 26)
                    if yq is not None:
                        ysb, colbase, q = yq
                        py = pypool.tile([DOUT, QW], fp32)
                        for k in range(NK):
                            nc.tensor.matmul(
                                py[:], wout_sb[:, k, :],
                                hist[:, k, colbase + q * QW:
                                     colbase + (q + 1) * QW],
                                start=(k == 0), stop=(k == NK - 1))
                        nc.vector.tensor_copy(ysb[:, q * QW:(q + 1) * QW], py[:])

                    if s == 6:   # window B of previous body -> yB (+BODY shift)
                        nc.sync.dma_start(
                            out=yb_d[:, ds(iv * NS + WIN * NS, WIN * NS)],
                            in_=ysbB[:])
                    if s == 31:  # window A of this body -> yA
                        nc.sync.dma_start(
                            out=ya_d[:, ds(iv * NS, WIN * NS)], in_=ysbA[:])

            # -------- post-loop: final window B (rows T-25..T-1) --------
            final_cur = hpsB if (BODY - 1) % 2 == 1 else hpsA
            nc.vector.tensor_scalar_mul(
                hist[:, :, (BODY - 1) * NS: BODY * NS], final_cur[:],
                1.0 - ALPHA)
            ysbF = ypool.tile([DOUT, WIN * NS], fp32, tag="ysbB")
            for q in range(4):
                py = pypool.tile([DOUT, QW], fp32)
                for k in range(NK):
                    nc.tensor.matmul(
                        py[:], wout_sb[:, k, :],
                        hist[:, k, WIN * NS + q * QW: WIN * NS + (q + 1) * QW],
                        start=(k == 0), stop=(k == NK - 1))
                nc.vector.tensor_copy(ysbF[:, q * QW:(q + 1) * QW], py[:])
            nc.sync.dma_start(
                out=yb_d[:, (t_total + WIN) * NS:(t_total + BODY) * NS],
                in_=ysbF[:])

    nc.compile()
    return nc


_NC_CACHE = {}


def _get_nc():
    if "nc" not in _NC_CACHE:
        _NC_CACHE["nc"] = _build_nc()
    return _NC_CACHE["nc"]


def _prep_in_maps(initdir, velocities, fc_w, fc_b, W_in, W_rec, W_out, bias):
    import ml_dtypes
    bfdt = ml_dtypes.bfloat16

    initdir = np.asarray(initdir, np.float32)
    velocities = np.asarray(velocities, np.float32)
    fc_w = np.asarray(fc_w, np.float32)
    fc_b = np.asarray(fc_b, np.float32)
    W_in = np.asarray(W_in, np.float32)
    W_rec = np.asarray(W_rec, np.float32)
    W_out = np.asarray(W_out, np.float32)
    bias = np.asarray(bias, np.float32)

    # host-side weight prep (shared across cores)
    wrec = (ALPHA * W_rec).reshape(NK, 128, H).astype(bfdt)
    win3 = (ALPHA * np.concatenate([W_in, bias[None, :]], axis=0)).astype(bfdt)
    wout = (W_out / (1.0 - ALPHA)).reshape(NK, 128, DOUT).astype(bfdt)
    fcw3 = np.concatenate([fc_w.T, fc_b[None, :]], axis=0)           # (3, H)

    in_maps = []
    for c in range(NCORES):
        sl = slice(c * NS, (c + 1) * NS)
        init3 = np.concatenate([initdir[sl].T,
                                np.ones((1, NS), np.float32)], axis=0)
        # xt[p, t*NS+n] = velocities[t, c*NS+n, p]; row DIN = ones
        xs = velocities[:, sl, :]                                    # (T, NS, 2)
        xt = np.empty((DIN + 1, T * NS), np.float32)
        xt[:DIN] = xs.transpose(2, 0, 1).reshape(DIN, T * NS)
        xt[DIN] = 1.0
        in_maps.append({
            "wrec": np.ascontiguousarray(wrec),
            "win3": np.ascontiguousarray(win3),
            "wout": np.ascontiguousarray(wout),
            "fcw3": np.ascontiguousarray(fcw3),
            "init3": np.ascontiguousarray(init3),
            "xt": xt.astype(bfdt),
        })
    return in_maps


def _unpack(res):
    out = np.empty((T, N, DOUT), np.float32)
    tmask = (np.arange(T) % BODY) < WIN
    for c in range(NCORES):
        ya = res.results[c]["ya"].reshape(DOUT, T, NS)
        yb = res.results[c]["yb"].reshape(DOUT, T + BODY, NS)
        yt = np.where(tmask[None, :, None], ya, yb[:, BODY:, :])
        out[:, c * NS:(c + 1) * NS, :] = yt.transpose(1, 2, 0)
    return out


def kernel(initdir, velocities, fc_w, fc_b, W_in, W_rec, W_out, bias):
    from concourse.bass_utils import run_bass_kernel_spmd

    in_maps = _prep_in_maps(initdir, velocities, fc_w, fc_b, W_in, W_rec,
                            W_out, bias)
    nc = _get_nc()
    res = run_bass_kernel_spmd(nc, in_maps, list(range(NCORES)))
    return _unpack(res)


# revision 7
# speedup vs baseline: 1.1143x; 1.1143x over previous
"""ContinuousTimeRNN Trainium2 kernel (v2: PSUM-resident h, bf16 matmuls).

Data-parallel over batch N=512 across 8 NeuronCores (NS=64 rows each).
h is kept transposed (H on partitions, [128, 4, 64]) and lives in PSUM.
Per step, one PSUM accumulation group computes

    h' = (0.9*I) @ h  +  sum_k (0.1*W_rec)[k].T @ a[k]  +  (0.1*[W_in;b]).T @ x3

where the 0.9*I matmul (fp32r, free=256 -> 1 cyc/row) opens the group
(start=True sets the PSUM has_written bits) and the 16 W_rec chunk matmuls
plus 4 x-chunk matmuls (bf16 moving tensors -> 1 cyc/row at any p-state)
accumulate on top.  The activation path reads PSUM directly:
ACT tanh(psum)->bf16, DVE max(.,0)->bf16, and DVE copies h to an SBUF
history ring (50 slots) that feeds both the 0.9*I matmul and the y output
matmuls (interleaved one per step into PE idle gaps).
"""

import sys

sys.path.insert(0, "/opt/trn_rl_repo")

import numpy as np

ALPHA = 0.1
T, N, H, DIN, DOUT, INIT = 1000, 512, 512, 2, 2, 2
NCORES = 8
NS = N // NCORES          # 64 batch rows per core
WIN = 25                  # y-flush window (steps)
BODY = 2 * WIN            # steps per For_i body
NK = H // 128             # 4 H-chunks
QW = WIN * NS // 4        # 400 columns per y quarter


def _build_nc(t_total=T, reps=1):
    import concourse.bass as bass
    import concourse.mybir as mybir
    from concourse import bacc
    from concourse.tile import TileContext
    from concourse.masks import make_identity
    from concourse.bass import ds

    fp32 = mybir.dt.float32
    fp32r = mybir.dt.float32r
    bf16 = mybir.dt.bfloat16
    fp16 = mybir.dt.float16
    AFT = mybir.ActivationFunctionType

    nc = bacc.Bacc("TRN2", target_bir_lowering=False, debug=False,
                   num_devices=NCORES)

    nbody = t_total // BODY

    # -------- DRAM I/O (per core) --------
    wrec_d = nc.dram_tensor("wrec", [NK, 128, H], fp16, kind="ExternalInput").ap()
    win3_d = nc.dram_tensor("win3", [DIN + 1, H], fp16, kind="ExternalInput").ap()
    wout_d = nc.dram_tensor("wout", [NK, 128, DOUT], fp16, kind="ExternalInput").ap()
    fcw3_d = nc.dram_tensor("fcw3", [INIT + 1, H], fp32, kind="ExternalInput").ap()
    init3_d = nc.dram_tensor("init3", [INIT + 1, NS], fp32, kind="ExternalInput").ap()
    xt_d = nc.dram_tensor("xt", [DIN + 1, t_total * NS], fp16, kind="ExternalInput").ap()
    # yA holds rows with (t % 50) < 25; yB holds the rest, shifted +BODY rows
    ya_d = nc.dram_tensor("ya", [DOUT, t_total * NS], fp32, kind="ExternalOutput").ap()
    yb_d = nc.dram_tensor("yb", [DOUT, (t_total + BODY) * NS], fp32,
                          kind="ExternalOutput").ap()

    with TileContext(nc) as tc:
        with (
            tc.tile_pool(name="wpool", bufs=1) as wpool,
            tc.tile_pool(name="hpool", bufs=1) as hpool,
            tc.tile_pool(name="apool", bufs=3) as apool,
            tc.tile_pool(name="xpool", bufs=2) as xpool,
            tc.tile_pool(name="ypool", bufs=2) as ypool,
            tc.tile_pool(name="hps", bufs=1, space="PSUM") as hpspool,
            tc.tile_pool(name="py", bufs=2, space="PSUM") as pypool,
        ):
            # -------- persistent SBUF --------
            wrec_sb = wpool.tile([128, NK, H], fp16)       # W_rec chunks (0.1 folded into a)
            win3_sb = wpool.tile([DIN + 1, H], fp16)       # 0.1*[W_in; bias]
            wout_sb = wpool.tile([128, NK, DOUT], fp16)    # W_out/0.9 chunks
            fcw3_sb = wpool.tile([INIT + 1, H], fp32)      # [fc_w.T; fc_b]
            init3_sb = wpool.tile([INIT + 1, NS], fp32)    # [initdir.T; ones]
            ident = wpool.tile([128, 128], fp16)           # identity (fp16)
            # h history ring (bf16, holds 0.9*h_after): slot s-1 feeds the
            # identity opener matmul of step s and the y matmuls (W_out/0.9)
            hist = hpool.tile([128, NK, BODY * NS], fp16)

            for k in range(NK):
                nc.sync.dma_start(out=wrec_sb[:, k, :], in_=wrec_d[k])
                nc.sync.dma_start(out=wout_sb[:, k, :], in_=wout_d[k])
            nc.sync.dma_start(out=win3_sb[:], in_=win3_d)
            nc.sync.dma_start(out=fcw3_sb[:], in_=fcw3_d)
            nc.sync.dma_start(out=init3_sb[:], in_=init3_d)
            make_identity(nc, ident[:])

            # persistent PSUM h tiles (ping-pong); h0 = fc(initdir) -> hpsB
            hpsA = hpspool.tile([128, NK, NS], fp32, tag="hpsA")
            hpsB = hpspool.tile([128, NK, NS], fp32, tag="hpsB")
            for m in range(NK):
                nc.tensor.matmul(hpsB[:, m, :],
                                 fcw3_sb[:, m * 128:(m + 1) * 128],
                                 init3_sb[:], start=True, stop=True)

            # -------- time loop --------
            with tc.For_i(0, reps, 1) as _rep, \
                 tc.For_i(0, t_total, BODY) as iv:
                xbuf = xpool.tile([DIN + 1, BODY * NS], fp16)
                nc.sync.dma_start(out=xbuf[:], in_=xt_d[:, ds(iv * NS, BODY * NS)])

                ysbB = ypool.tile([DOUT, WIN * NS], fp32, tag="ysbB")
                ysbA = ypool.tile([DOUT, WIN * NS], fp32, tag="ysbA")

                # y matmuls pending per step: (ysb, colbase, q) emitted one
                # PE-accumulation-quarter (4 matmuls) per step.
                # s=1..4   : prev-body window B (hist slots 25..49)
                # s=26..29 : this-body window A (hist slots 0..24)
                for s in range(BODY):
                    cur, prev = (hpsA, hpsB) if s % 2 == 0 else (hpsB, hpsA)
                    slot = (s - 1) % BODY

                    # 0.9*h_after(s-1) -> hist ring (bf16)
                    nc.vector.tensor_scalar_mul(
                        hist[:, :, slot * NS:(slot + 1) * NS], prev[:],
                        1.0 - ALPHA)
                    # a = max(tanh(h), 0) in bf16
                    tbuf = apool.tile([128, NK * NS], fp16, tag="tbuf")
                    abuf = apool.tile([128, NK, NS], fp16, tag="abuf")
                    nc.scalar.activation(tbuf[:], prev[:].rearrange(
                        "p k n -> p (k n)"), AFT.Tanh)
                    nc.vector.tensor_scalar(
                        out=abuf[:],
                        in0=tbuf[:].rearrange("p (k n) -> p k n", k=NK),
                        scalar1=0.0, scalar2=ALPHA,
                        op0=mybir.AluOpType.max, op1=mybir.AluOpType.mult)

                    # h' accumulation group: 0.9*I opener + 16 aW + 4 x
                    nc.tensor.matmul(
                        cur[:],
                        ident[:],
                        hist[:, :, slot * NS:(slot + 1) * NS],
                        start=True, stop=False, skip_group_check=True)
                    for k in range(NK):
                        for m in range(NK):
                            nc.tensor.matmul(
                                cur[:, m, :],
                                wrec_sb[:, k, m * 128:(m + 1) * 128],
                                abuf[:, k, :],
                                start=False, stop=False, skip_group_check=True)
                    for m in range(NK):
                        nc.tensor.matmul(
                            cur[:, m, :],
                            win3_sb[:, m * 128:(m + 1) * 128],
                            xbuf[:, s * NS:(s + 1) * NS],
                            start=False, stop=True, skip_group_check=True)

                    # interleaved y work (one PSUM quarter per step)
                    yq = None
                    if 1 <= s <= 4:        # prev-body window B: slots 25..49
                        yq = (ysbB, WIN * NS, s - 1)
                    elif 26 <= s <= 29:    # this-body window A: slots 0..24
                        yq = (ysbA, 0, s - 26)
                    if yq is not None:
                        ysb, colbase, q = yq
                        py = pypool.tile([DOUT, QW], fp32)
                        for k in range(NK):
                            nc.tensor.matmul(
                                py[:], wout_sb[:, k, :],
                                hist[:, k, colbase + q * QW:
                                     colbase + (q + 1) * QW],
                                start=(k == 0), stop=(k == NK - 1))
                        nc.vector.tensor_copy(ysb[:, q * QW:(q + 1) * QW], py[:])

                    if s == 6:   # window B of previous body -> yB (+BODY shift)
                        nc.sync.dma_start(
                            out=yb_d[:, ds(iv * NS + WIN * NS, WIN * NS)],
                            in_=ysbB[:])
                    if s == 31:  # window A of this body -> yA
                        nc.sync.dma_start(
                            out=ya_d[:, ds(iv * NS, WIN * NS)], in_=ysbA[:])

            # -------- post-loop: final window B (rows T-25..T-1) --------
            final_cur = hpsB if (BODY - 1) % 2 == 1 else hpsA
            nc.vector.tensor_scalar_mul(
                hist[:, :, (BODY - 1) * NS: BODY * NS], final_cur[:],
                1.0 - ALPHA)
            ysbF = ypool.tile([DOUT, WIN * NS], fp32, tag="ysbB")
            for q in range(4):
                py = pypool.tile([DOUT, QW], fp32)
                for k in range(NK):
                    nc.tensor.matmul(
                        py[:], wout_sb[:, k, :],
                        hist[:, k, WIN * NS + q * QW: WIN * NS + (q + 1) * QW],
                        start=(k == 0), stop=(k == NK - 1))
                nc.vector.tensor_copy(ysbF[:, q * QW:(q + 1) * QW], py[:])
            nc.sync.dma_start(
                out=yb_d[:, (t_total + WIN) * NS:(t_total + BODY) * NS],
                in_=ysbF[:])

    nc.compile()
    return nc


_NC_CACHE = {}


def _get_nc():
    if "nc" not in _NC_CACHE:
        _NC_CACHE["nc"] = _build_nc()
    return _NC_CACHE["nc"]


def _prep_in_maps(initdir, velocities, fc_w, fc_b, W_in, W_rec, W_out, bias):
    initdir = np.asarray(initdir, np.float32)
    velocities = np.asarray(velocities, np.float32)
    fc_w = np.asarray(fc_w, np.float32)
    fc_b = np.asarray(fc_b, np.float32)
    W_in = np.asarray(W_in, np.float32)
    W_rec = np.asarray(W_rec, np.float32)
    W_out = np.asarray(W_out, np.float32)
    bias = np.asarray(bias, np.float32)

    # host-side weight prep (shared across cores)
    wrec = W_rec.reshape(NK, 128, H).astype(np.float16)
    win3 = (ALPHA * np.concatenate([W_in, bias[None, :]], axis=0)).astype(np.float16)
    wout = (W_out / (1.0 - ALPHA)).reshape(NK, 128, DOUT).astype(np.float16)
    fcw3 = np.concatenate([fc_w.T, fc_b[None, :]], axis=0)           # (3, H)

    in_maps = []
    for c in range(NCORES):
        sl = slice(c * NS, (c + 1) * NS)
        init3 = np.concatenate([initdir[sl].T,
                                np.ones((1, NS), np.float32)], axis=0)
        # xt[p, t*NS+n] = velocities[t, c*NS+n, p]; row DIN = ones
        xs = velocities[:, sl, :]                                    # (T, NS, 2)
        xt = np.empty((DIN + 1, T * NS), np.float32)
        xt[:DIN] = xs.transpose(2, 0, 1).reshape(DIN, T * NS)
        xt[DIN] = 1.0
        in_maps.append({
            "wrec": np.ascontiguousarray(wrec),
            "win3": np.ascontiguousarray(win3),
            "wout": np.ascontiguousarray(wout),
            "fcw3": np.ascontiguousarray(fcw3),
            "init3": np.ascontiguousarray(init3),
            "xt": xt.astype(np.float16),
        })
    return in_maps


def _unpack(res):
    out = np.empty((T, N, DOUT), np.float32)
    tmask = (np.arange(T) % BODY) < WIN
    for c in range(NCORES):
        ya = res.results[c]["ya"].reshape(DOUT, T, NS)
        yb = res.results[c]["yb"].reshape(DOUT, T + BODY, NS)
        yt = np.where(tmask[None, :, None], ya, yb[:, BODY:, :])
        out[:, c * NS:(c + 1) * NS, :] = yt.transpose(1, 2, 0)
    return out


def kernel(initdir, velocities, fc_w, fc_b, W_in, W_rec, W_out, bias):
    from concourse.bass_utils import run_bass_kernel_spmd

    in_maps = _prep_in_maps(initdir, velocities, fc_w, fc_b, W_in, W_rec,
                            W_out, bias)
    nc = _get_nc()
    res = run_bass_kernel_spmd(nc, in_maps, list(range(NCORES)))
    return _unpack(res)
